# revision 14
# baseline (speedup 1.0000x reference)
"""Trainium2 Bass kernel for nn_ExpertClassifierBank.

Computes, for pooled [B,K,D], expert weights [E,C,D], indices [K], log_scales [E]:
    x = l2norm(pooled, axis=-1)
    w = l2norm(weights[idx], axis=-1)
    out[b,k,c] = min(exp(log_scales[idx[k]]), 100) * dot(x[b,k], w[k,c])

Sharding: data-parallel over batch B across 8 NeuronCores (512 rows each);
the gathered expert weight bank is replicated.

Device algorithm per core (B_loc=512, K=8, D=1024=8x128, C=100):
  - host pre-transposes x and w to [k, d-part, j, *] bf16 tiles so the
    contraction dim d sits on SBUF partitions; w is host-scaled by 32 (scale
    cancels in the cosine) so w^2 fits fp8-e4m3 range.
  - main matmuls use the raw bf16 tiles; both cosine normalizers are applied
    at output time:  out[c,b] = lgs[c,b] * comb[c,b]  where
    comb = rw[k,c] * f[b] comes from ONE K=4 matmul per k whose lhsT is a
    zero-masked [4,C] tile holding rw[k] in row k%4 and whose rhs is the
    half's f tile [4,BLOC]; lgs is the raw ACT drain of the logit PSUM.
  - row sums-of-squares stream through the PE as fp8-e4m3 squares with
    perf_mode=DoubleRow (d-chunk pairs on the 256-deep virtual array), halving
    the second x pass; selectors put each k's sum in its half-slot partition.
  - the whole scalar chain uses one ACT table set (sqrt_and_others:
    Square / Sqrt / Copy); reciprocals ride the DVE; exp(log_scales) folds
    into a host-computed scale^2 that rides the Sqrt input scale.
  - squares are split ACT/DVE/GPSIMD per d-chunk pair; drains run one k
    behind the matmul stream, the comb/mul/out stage four k behind (gated by
    the half's f tile).
  - x halves stream on the sync HWDGE queue from t=0; w rides the gpsimd
    SWDGE queue so wss closes early; outputs k<4 go out mid-kernel on the
    scalar queue, k>=4 on the sync queue tail.
"""

import time

import numpy as np
import ml_dtypes

import concourse.bass as bass
import concourse.mybir as mybir
import concourse.tile as tile
from concourse import bacc
from concourse.bass_utils import run_bass_kernel_spmd

N_CORES = 8
B, K, D, C, E = 4096, 8, 1024, 100, 16
BLOC = B // N_CORES  # 512
P = 128
DC = D // P  # 8 d-chunks
HALF = 4  # k-batch size for the f pipeline
NPAIR = DC // 2  # d-chunk pairs for DoubleRow

F32 = mybir.dt.float32
F32R = mybir.dt.float32r
BF16 = mybir.dt.bfloat16
FP8 = mybir.dt.float8e4
AF = mybir.ActivationFunctionType
DR = mybir.MatmulPerfMode.DoubleRow
NPBF16 = ml_dtypes.bfloat16
NPFP8 = ml_dtypes.float8_e4m3

_CACHE = {}

LAST_RESULT = None
LAST_WALL_NS = None


def _build():
    nc = bacc.Bacc(
        "TRN2", target_bir_lowering=False, debug=False, num_devices=N_CORES
    )

    xt = nc.dram_tensor("xt", [K, P, DC, BLOC], BF16, kind="ExternalInput").ap()
    wt = nc.dram_tensor("wt", [K, P, DC, C], BF16, kind="ExternalInput").ap()
    s2h = nc.dram_tensor("s2h", [HALF, 2], F32, kind="ExternalInput").ap()
    selx = nc.dram_tensor("selx", [P, 2, 8 * HALF], FP8, kind="ExternalInput").ap()
    mask4 = nc.dram_tensor("mask4", [HALF, HALF, C], F32, kind="ExternalInput").ap()
    out = nc.dram_tensor("out", [K, C, BLOC], BF16, kind="ExternalOutput").ap()

    with tile.TileContext(nc) as tc:
        with (
            tc.tile_pool(name="const", bufs=1) as cpool,
            tc.tile_pool(name="xres", bufs=K) as xpool,
            tc.tile_pool(name="x2", bufs=3) as x2pool,
            tc.tile_pool(name="wres", bufs=K) as wpool,
            tc.tile_pool(name="w2", bufs=2) as w2pool,
            tc.tile_pool(name="small", bufs=1) as spool,
            tc.tile_pool(name="lgs", bufs=5) as lgspool,
            tc.tile_pool(name="osb", bufs=3) as opool,
            tc.tile_pool(name="fx", bufs=2) as fxpool,
        ):
            # ---- consts on the gpsimd SWDGE queue ----
            s2h_sb = cpool.tile([HALF, 2], F32)
            nc.gpsimd.dma_start(s2h_sb[:], s2h[:])
            selx_sb = cpool.tile([P, 2, 8 * HALF], FP8)
            nc.gpsimd.dma_start(selx_sb[:], selx[:])
            mask4_sb = cpool.tile([HALF, HALF, C], F32)
            nc.gpsimd.dma_start(mask4_sb[:], mask4[:])

            # ---- bulk DMAs: x halves on sync HWDGE, w on gpsimd SWDGE ----
            x_sbs = []
            for k in range(K):
                x_sb = xpool.tile([P, DC, BLOC], BF16, tag="x", name=f"x{k}")
                nc.sync.dma_start(x_sb[:, : DC // 2], xt[k][:, : DC // 2])
                nc.sync.dma_start(x_sb[:, DC // 2 :], xt[k][:, DC // 2 :])
                x_sbs.append(x_sb)
            w_sbs = []
            for k in range(K):
                w1 = wpool.tile([P, DC, C], BF16, tag="w", name=f"w{k}")
                nc.gpsimd.dma_start(w1[:], wt[k])
                w_sbs.append(w1)

            # rw masked tiles: row k%4 holds rw[k] = scale/||w_k||, rest 0
            rwm_sbs = [None] * K

            with (
                tc.tile_pool(name="pwss", bufs=1, space="PSUM") as pwss,
                tc.tile_pool(name="pss", bufs=2, space="PSUM") as pss,
                tc.tile_pool(name="plog", bufs=3, space="PSUM") as plog,
                tc.tile_pool(name="pcomb", bufs=2, space="PSUM") as pcomb,
            ):
                # wss2[:, 100h:100h+100]: half h's ||w||^2 by k-slot
                wss2 = pwss.tile([HALF, 2 * C], F32)

                sss = []
                fx_sbs = []
                lg_tiles = {}
                o_done = set()

                # square-pair engine split per k: [DVE, ACT, GP, DVE/GP]
                def sq_engine(k, p):
                    if p == 0:
                        return nc.vector
                    if p == 1:
                        return None  # ACT (activation Square)
                    if p == 2:
                        return nc.gpsimd
                    return nc.vector if k % 2 == 0 else nc.gpsimd

                def emit_drain(k):
                    lgs = lgspool.tile([C, BLOC], F32, tag="lgs",
                                       name=f"lgs{k}")
                    nc.scalar.activation(lgs[:], lg_tiles[k][:], AF.Copy)
                    lg_tiles[k] = lgs  # now points at the SBUF copy

                def emit_output_stage(k):
                    """comb matmul + mul + out DMA for k (fx half ready)."""
                    half = k // HALF
                    comb = pcomb.tile([C, BLOC], F32, tag="comb",
                                      name=f"comb{k}")
                    nc.tensor.matmul(
                        comb[:],
                        lhsT=rwm_sbs[k][:],
                        rhs=fx_sbs[half][:],
                        start=True, stop=True,
                        skip_group_check=True,
                    )
                    o_sb = opool.tile([C, BLOC], BF16, tag="osb", name=f"o{k}")
                    nc.vector.tensor_mul(o_sb[:], lg_tiles[k][:], comb[:])
                    if k < HALF:
                        nc.scalar.dma_start(out[k], o_sb[:])
                    else:
                        nc.sync.dma_start(out[k], o_sb[:])
                    o_done.add(k)

                for k in range(K):
                    half, i = divmod(k, HALF)
                    if i == 0:
                        ss = pss.tile([HALF, BLOC], F32, tag="ss",
                                      name=f"ss{half}")
                        sss.append(ss)
                    ss = sss[half]

                    # ---- squares: x^2 in fp8 by d-chunk pair ----
                    x2 = x2pool.tile([P, DC, BLOC], FP8, tag="x2",
                                     name=f"x2_{k}")
                    for p in range(NPAIR):
                        src = x_sbs[k][:, 2 * p : 2 * p + 2]
                        dst = x2[:, 2 * p : 2 * p + 2]
                        eng = sq_engine(k, p)
                        if eng is None:
                            nc.scalar.activation(dst, src, AF.Square)
                        else:
                            eng.tensor_mul(dst, src, src)

                    # ---- w^2 (fp8) + wss DoubleRow matmuls ----
                    w2 = w2pool.tile([P, DC, C], FP8, tag="w2", name=f"w2_{k}")
                    nc.scalar.activation(w2[:], w_sbs[k][:], AF.Square)
                    for p in range(NPAIR):
                        nc.tensor.matmul(
                            wss2[:, C * half : C * half + C],
                            lhsT=selx_sb[:, :, 8 * i : 8 * i + HALF],
                            rhs=w2[:, 2 * p : 2 * p + 2],
                            start=(i == 0 and p == 0),
                            stop=(i == HALF - 1 and p == NPAIR - 1),
                            perf_mode=DR,
                            skip_group_check=True,
                        )

                    # ---- ss DoubleRow matmuls ----
                    for p in range(NPAIR):
                        nc.tensor.matmul(
                            ss[:],
                            lhsT=selx_sb[:, :, 8 * i : 8 * i + HALF],
                            rhs=x2[:, 2 * p : 2 * p + 2],
                            start=(i == 0 and p == 0),
                            stop=(i == HALF - 1 and p == NPAIR - 1),
                            perf_mode=DR,
                            skip_group_check=True,
                        )

                    # ---- main matmuls ----
                    lg = plog.tile([C, BLOC], F32, tag="lg", name=f"lg{k}")
                    for j in range(DC):
                        nc.tensor.matmul(
                            lg[:],
                            lhsT=w_sbs[k][:, j, :],
                            rhs=x_sbs[k][:, j],
                            start=(j == 0),
                            stop=(j == DC - 1),
                            skip_group_check=True,
                        )
                    lg_tiles[k] = lg

                    if k >= 1:
                        emit_drain(k - 1)

                    # half closed: rw chain + f tile
                    # rw = sqrt(s^2 / ||w||^2); f = sqrt(1/ss)
                    if i == HALF - 1:
                        recw = spool.tile([HALF, C], F32, name=f"recw{half}")
                        nc.vector.reciprocal(
                            recw[:], wss2[:, C * half : C * half + C]
                        )
                        rwh = spool.tile([HALF, C], F32, name=f"rwh{half}")
                        nc.scalar.activation(
                            rwh[:], recw[:], AF.Sqrt,
                            scale=s2h_sb[:, half : half + 1],
                        )
                        for ii in range(HALF):
                            kk = half * HALF + ii
                            rwm = spool.tile([HALF, C], F32R, name=f"rwm{kk}")
                            nc.vector.tensor_mul(
                                rwm[:], rwh[:], mask4_sb[:, ii, :]
                            )
                            rwm_sbs[kk] = rwm
                        recx = fxpool.tile([HALF, BLOC], F32, tag="recx",
                                           name=f"recx{half}")
                        scr = fxpool.tile([HALF, BLOC], F32, tag="rscr",
                                          name=f"rscr{half}")
                        nc.vector.reciprocal_approx_accurate(
                            recx[:], ss[:], scr[:]
                        )
                        fx = fxpool.tile([HALF, BLOC], F32R, tag="fx",
                                         name=f"fx{half}")
                        nc.scalar.activation(fx[:], recx[:], AF.Sqrt)
                        fx_sbs.append(fx)

                    if k >= HALF:
                        emit_output_stage(k - HALF)

                emit_drain(K - 1)
                for k in range(K):
                    if k not in o_done:
                        emit_output_stage(k)

    nc.compile()
    return nc


def _host_prep(pooled, active_expert_indices, weights, log_scales):
    idx = np.asarray(active_expert_indices).astype(np.int64)
    pooled = np.asarray(pooled, dtype=np.float32)
    weights = np.asarray(weights, dtype=np.float32)
    log_scales = np.asarray(log_scales, dtype=np.float32)

    # x: [B,K,D] -> bf16 -> per-core [K, P, DC, BLOC]  (k, d, j, b)
    pb = pooled.astype(NPBF16)
    xt_all = np.ascontiguousarray(
        pb.reshape(N_CORES, BLOC, K, DC, P).transpose(0, 2, 4, 3, 1)
    )
    # w: gather -> x32 (cancels in cosine; keeps w^2 in fp8 range) -> bf16
    wg = (32.0 * weights[idx]).astype(NPBF16)
    wt = np.ascontiguousarray(wg.reshape(K, C, DC, P).transpose(0, 3, 2, 1))

    # scale^2 arranged [i, half] so it rides the Sqrt input scale
    s = np.minimum(np.exp(log_scales[idx]), 100.0)
    s2h = np.ascontiguousarray((s * s).reshape(2, HALF).T.astype(np.float32))

    # DoubleRow ss selector: block i at cols 8i..8i+7, one-hot col i
    selx = np.zeros((P, 2, 8 * HALF), NPFP8)
    for i in range(HALF):
        selx[:, :, 8 * i + i] = 1.0
    # one-hot row masks for the per-k rw tiles
    mask4 = np.zeros((HALF, HALF, C), np.float32)
    for i in range(HALF):
        mask4[i, i, :] = 1.0

    shared = {"wt": wt, "s2h": s2h, "selx": selx, "mask4": mask4}
    return [dict(shared, xt=np.ascontiguousarray(xt_all[co]))
            for co in range(N_CORES)]


def kernel(pooled, active_expert_indices, weights, log_scales):
    global LAST_RESULT, LAST_WALL_NS
    if "nc" not in _CACHE:
        _CACHE["nc"] = _build()
    nc = _CACHE["nc"]

    in_maps = _host_prep(pooled, active_expert_indices, weights, log_scales)

    t0 = time.perf_counter_ns()
    res = run_bass_kernel_spmd(nc, in_maps, core_ids=list(range(N_CORES)))
    LAST_WALL_NS = time.perf_counter_ns() - t0
    LAST_RESULT = res

    full = np.stack([res.results[co]["out"] for co in range(N_CORES)])
    return np.ascontiguousarray(
        full.transpose(0, 3, 1, 2).reshape(B, K, C)
    ).astype(np.float32)


# revision 15
# speedup vs baseline: 1.0825x; 1.0825x over previous
"""Trainium2 Bass kernel for nn_ExpertClassifierBank.

Computes, for pooled [B,K,D], expert weights [E,C,D], indices [K], log_scales [E]:
    x = l2norm(pooled, axis=-1)
    w = l2norm(weights[idx], axis=-1)
    out[b,k,c] = min(exp(log_scales[idx[k]]), 100) * dot(x[b,k], w[k,c])

Sharding: data-parallel over batch B across 8 NeuronCores (512 rows each);
the gathered expert weight bank is replicated.

Device algorithm per core (B_loc=512, K=8, D=1024=8x128, C=100):
  - host pre-transposes x and w to [k, d-part, j, *] bf16 tiles so the
    contraction dim d sits on SBUF partitions for the main matmuls; w is
    ALSO shipped in its natural gathered layout [k, c-part, d-free] so
    ||w||^2 comes from ONE ACT Square-with-accum per k -- no PE pass, and
    the result lands C-on-partitions, exactly the layout the logit drain
    scale needs.  rw = sqrt(1/wss * s^2) rides tiny per-k DVE/ACT ops.
  - main matmuls produce lg[k] = w_k^T x in PSUM; the drain to SBUF applies
    the w-normalizer as a per-partition scale (ACT Copy, scale=rwt[:,k]);
    the x-normalizer f = rsqrt(||x||^2) is broadcast over C partitions by a
    tiny selector matmul (selc4, f32r) and multiplied in on the DVE with
    bf16 output that goes straight to HBM (halves output traffic).
  - row sums-of-squares stream bf16 squares through the PE with one-hot
    selectors so each k's ss lands in its half-slot partition; squares are
    split DVE (2x packed bf16) / ACT per d-chunk pair.
  - one ACT table set (sqrt_and_others: Square/Sqrt/Copy) loaded at t~0 by
    a dummy op; exp(log_scales) folds into host-computed s^2 tensors.
  - x halves stream on the sync HWDGE queue from t=0; both w copies ride
    the gpsimd SWDGE queue; outputs k<4 go out mid-kernel on gpsimd, k>=4
    on the sync queue tail.  Drains run one k behind the matmul stream, the
    f/output stage four k behind (gated by the half's f tile).
"""

import time

import numpy as np
import ml_dtypes

import concourse.bass as bass
import concourse.mybir as mybir
import concourse.tile as tile
from concourse import bacc
from concourse.bass_utils import run_bass_kernel_spmd

N_CORES = 8
B, K, D, C, E = 4096, 8, 1024, 100, 16
BLOC = B // N_CORES  # 512
P = 128
DC = D // P  # 8 d-chunks
HALF = 4  # k-batch size for the f pipeline
NPAIR = DC // 2

F32 = mybir.dt.float32
F32R = mybir.dt.float32r
BF16 = mybir.dt.bfloat16
AF = mybir.ActivationFunctionType
NPBF16 = ml_dtypes.bfloat16

_CACHE = {}

LAST_RESULT = None
LAST_WALL_NS = None


def _build():
    nc = bacc.Bacc(
        "TRN2", target_bir_lowering=False, debug=False, num_devices=N_CORES
    )

    xt = nc.dram_tensor("xt", [K, P, DC, BLOC], BF16, kind="ExternalInput").ap()
    wt = nc.dram_tensor("wt", [K, P, DC, C], BF16, kind="ExternalInput").ap()
    wq = nc.dram_tensor("wq", [K, C, D], BF16, kind="ExternalInput").ap()
    s2bc = nc.dram_tensor("s2bc", [C, K], F32, kind="ExternalInput").ap()
    selk4 = nc.dram_tensor("selk4", [P, HALF, HALF], BF16, kind="ExternalInput").ap()
    selc4 = nc.dram_tensor("selc4", [HALF, HALF, C], F32R, kind="ExternalInput").ap()
    out = nc.dram_tensor("out", [K, C, BLOC], BF16, kind="ExternalOutput").ap()

    with tile.TileContext(nc) as tc:
        with (
            tc.tile_pool(name="const", bufs=1) as cpool,
            tc.tile_pool(name="xres", bufs=K) as xpool,
            tc.tile_pool(name="x2", bufs=3) as x2pool,
            tc.tile_pool(name="wres", bufs=K) as wpool,
            tc.tile_pool(name="wqres", bufs=2) as wqpool,
            tc.tile_pool(name="w2junk", bufs=2) as w2pool,
            tc.tile_pool(name="small", bufs=1) as spool,
            tc.tile_pool(name="lgs", bufs=5) as lgspool,
            tc.tile_pool(name="osb", bufs=3) as opool,
            tc.tile_pool(name="fx", bufs=2) as fxpool,
        ):
            # dummy first ACT op: pulls the (single) table set load to t~0
            dum = spool.tile([1, 1], F32, name="dum")
            nc.vector.memset(dum[:], 1.0)
            dum2 = spool.tile([1, 1], F32, name="dum2")
            nc.scalar.activation(dum2[:], dum[:], AF.Square)

            # ---- consts + both w copies on the gpsimd SWDGE queue ----
            selk4_sb = cpool.tile([P, HALF, HALF], BF16)
            nc.gpsimd.dma_start(selk4_sb[:], selk4[:])
            s2bc_sb = cpool.tile([C, K], F32)
            nc.gpsimd.dma_start(s2bc_sb[:], s2bc[:])
            w_sbs = []
            wq_sbs = []
            for k in range(K):
                w1 = wpool.tile([P, DC, C], BF16, tag="w", name=f"w{k}")
                nc.gpsimd.dma_start(w1[:], wt[k])
                w_sbs.append(w1)
                wq1 = wqpool.tile([C, D], BF16, tag="wq", name=f"wq{k}")
                nc.gpsimd.dma_start(wq1[:], wq[k])
                wq_sbs.append(wq1)
                if k == 1:
                    selc4_sb = cpool.tile([HALF, HALF, C], F32R)
                    nc.gpsimd.dma_start(selc4_sb[:], selc4[:])

            # ---- x halves on the sync HWDGE queue ----
            x_sbs = []
            for k in range(K):
                x_sb = xpool.tile([P, DC, BLOC], BF16, tag="x", name=f"x{k}")
                nc.sync.dma_start(x_sb[:, : DC // 2], xt[k][:, : DC // 2])
                nc.sync.dma_start(x_sb[:, DC // 2 :], xt[k][:, DC // 2 :])
                x_sbs.append(x_sb)

            # wss/rw columns, [C-part, K]
            wss_sb = spool.tile([C, K], F32, name="wss")
            recw_sb = spool.tile([C, K], F32, name="recw")
            rwt_sb = spool.tile([C, K], F32, name="rwt")

            with (
                tc.tile_pool(name="pss", bufs=2, space="PSUM") as pss,
                tc.tile_pool(name="plog", bufs=3, space="PSUM") as plog,
                tc.tile_pool(name="pfb", bufs=2, space="PSUM") as pfb,
            ):
                sss = []
                fx_sbs = []
                lg_tiles = {}
                o_done = set()

                def emit_drain(k):
                    lgs = lgspool.tile([C, BLOC], F32, tag="lgs",
                                       name=f"lgs{k}")
                    nc.scalar.activation(
                        lgs[:], lg_tiles[k][:], AF.Copy,
                        scale=rwt_sb[:, k : k + 1],
                    )
                    lg_tiles[k] = lgs

                def emit_output_stage(k):
                    """f-broadcast matmul + mul + out DMA (fx half ready)."""
                    half, i = divmod(k, HALF)
                    fb = pfb.tile([C, BLOC], F32, tag="fb", name=f"fb{k}")
                    nc.tensor.matmul(
                        fb[:],
                        lhsT=selc4_sb[:, i, :],
                        rhs=fx_sbs[half][:],
                        start=True, stop=True,
                        skip_group_check=True,
                    )
                    o_sb = opool.tile([C, BLOC], BF16, tag="osb", name=f"o{k}")
                    nc.vector.tensor_mul(o_sb[:], lg_tiles[k][:], fb[:])
                    if k < HALF:
                        nc.gpsimd.dma_start(out[k], o_sb[:])
                    else:
                        nc.sync.dma_start(out[k], o_sb[:])
                    o_done.add(k)

                for k in range(K):
                    half, i = divmod(k, HALF)
                    if i == 0:
                        ss = pss.tile([HALF, BLOC], F32, tag="ss",
                                      name=f"ss{half}")
                        sss.append(ss)
                    ss = sss[half]

                    # ---- squares: bf16, pairs split DVE / ACT ----
                    x2 = x2pool.tile([P, DC, BLOC], BF16, tag="x2",
                                     name=f"x2_{k}")
                    for p in range(NPAIR):
                        src = x_sbs[k][:, 2 * p : 2 * p + 2]
                        dst = x2[:, 2 * p : 2 * p + 2]
                        if p % 2 == 0:
                            nc.vector.tensor_mul(dst, src, src)
                        else:
                            nc.scalar.activation(dst, src, AF.Square)

                    # ---- w-branch: ACT square-with-accum + tiny rw chain ----
                    w2j = w2pool.tile([C, D], BF16, tag="w2j", name=f"w2j{k}")
                    nc.scalar.activation(
                        w2j[:], wq_sbs[k][:], AF.Square,
                        accum_out=wss_sb[:, k : k + 1],
                    )
                    nc.vector.reciprocal(
                        recw_sb[:, k : k + 1], wss_sb[:, k : k + 1]
                    )
                    # rw = sqrt(s^2 / wss)
                    nc.scalar.activation(
                        rwt_sb[:, k : k + 1], recw_sb[:, k : k + 1], AF.Sqrt,
                        scale=s2bc_sb[:, k : k + 1],
                    )

                    # ---- ss matmuls (one-hot k-slot selector) ----
                    for j in range(DC):
                        nc.tensor.matmul(
                            ss[:],
                            lhsT=selk4_sb[:, i, :],
                            rhs=x2[:, j],
                            start=(i == 0 and j == 0),
                            stop=(i == HALF - 1 and j == DC - 1),
                            skip_group_check=True,
                        )

                    # ---- main matmuls ----
                    lg = plog.tile([C, BLOC], F32, tag="lg", name=f"lg{k}")
                    for j in range(DC):
                        nc.tensor.matmul(
                            lg[:],
                            lhsT=w_sbs[k][:, j, :],
                            rhs=x_sbs[k][:, j],
                            start=(j == 0),
                            stop=(j == DC - 1),
                            skip_group_check=True,
                        )
                    lg_tiles[k] = lg

                    if k >= 1:
                        emit_drain(k - 1)

                    # half closed: f = sqrt(1/ss)
                    if i == HALF - 1:
                        recx = fxpool.tile([HALF, BLOC], F32, tag="recx",
                                           name=f"recx{half}")
                        scr = fxpool.tile([HALF, BLOC], F32, tag="rscr",
                                          name=f"rscr{half}")
                        nc.vector.reciprocal_approx_accurate(
                            recx[:], ss[:], scr[:]
                        )
                        fx = fxpool.tile([HALF, BLOC], F32R, tag="fx",
                                         name=f"fx{half}")
                        nc.scalar.activation(fx[:], recx[:], AF.Sqrt)
                        fx_sbs.append(fx)

                    if k >= HALF:
                        emit_output_stage(k - HALF)

                emit_drain(K - 1)
                for k in range(K):
                    if k not in o_done:
                        emit_output_stage(k)

    nc.compile()
    return nc


def _host_prep(pooled, active_expert_indices, weights, log_scales):
    idx = np.asarray(active_expert_indices).astype(np.int64)
    pooled = np.asarray(pooled, dtype=np.float32)
    weights = np.asarray(weights, dtype=np.float32)
    log_scales = np.asarray(log_scales, dtype=np.float32)

    # x: [B,K,D] -> bf16 -> per-core [K, P, DC, BLOC]  (k, d, j, b)
    pb = pooled.astype(NPBF16)
    xt_all = np.ascontiguousarray(
        pb.reshape(N_CORES, BLOC, K, DC, P).transpose(0, 2, 4, 3, 1)
    )
    # w: gather -> bf16, twice: matmul layout + natural layout for ||w||^2
    wg = weights[idx].astype(NPBF16)  # [K, C, D]
    wt = np.ascontiguousarray(wg.reshape(K, C, DC, P).transpose(0, 3, 2, 1))
    wq = np.ascontiguousarray(wg)

    # scale^2 broadcast down C partitions: rides the rw Sqrt input scale
    s = np.minimum(np.exp(log_scales[idx]), 100.0).astype(np.float32)
    s2bc = np.ascontiguousarray(np.tile((s * s)[None, :], (C, 1)))

    selk4 = np.zeros((P, HALF, HALF), NPBF16)
    for i in range(HALF):
        selk4[:, i, i] = 1.0
    selc4 = np.zeros((HALF, HALF, C), np.float32)
    for i in range(HALF):
        selc4[i, i, :] = 1.0

    shared = {"wt": wt, "wq": wq, "s2bc": s2bc, "selk4": selk4,
              "selc4": selc4}
    return [dict(shared, xt=np.ascontiguousarray(xt_all[co]))
            for co in range(N_CORES)]


def kernel(pooled, active_expert_indices, weights, log_scales):
    global LAST_RESULT, LAST_WALL_NS
    if "nc" not in _CACHE:
        _CACHE["nc"] = _build()
    nc = _CACHE["nc"]

    in_maps = _host_prep(pooled, active_expert_indices, weights, log_scales)

    t0 = time.perf_counter_ns()
    res = run_bass_kernel_spmd(nc, in_maps, core_ids=list(range(N_CORES)))
    LAST_WALL_NS = time.perf_counter_ns() - t0
    LAST_RESULT = res

    full = np.stack([res.results[co]["out"] for co in range(N_CORES)])
    return np.ascontiguousarray(
        full.transpose(0, 3, 1, 2).reshape(B, K, C)
    ).astype(np.float32)
